# revision 25
# baseline (speedup 1.0000x reference)
"""Contrastive (NT-Xent) loss kernel for 8 Trainium2 NeuronCores.

Math (reference): z = l2norm(concat(proj_1, proj_2)) [8192,128];
sim = z @ z.T; loss = mean_i( log(sum_{j!=i} exp(2*sim_ij)) - 2*pos_i ).

Sharding: rows of the 8192x8192 sim matrix are split 1024/core. Each core
receives the full rep matrix rotated by core*1024 rows (host-side layout
only), so its own rows are always local columns [0,1024) and the positive
partners are at [4096,5120) -- one identical SPMD program, static offsets.
Each core emits one partial scalar; the host sums 8 floats.

Device pipeline (per core), interleaved in groups of 2048 columns so the
ACT engine (the bottleneck: exp at 1 elem/cycle/lane) starts ~5us in:
  group g: DMA 4 natural bf16 chunks -> row norms (DVE, fp32 accum) ->
  1/norm via ln/exp (compact [128,16], ACT) -> per-row scale (DVE) ->
  PE-transpose into normalized bf16 X^T columns -> main quarter g:
  8 m-tiles x (4 bf16 matmuls -> fused exp+row-sum ACT op, [128,2048]
  PSUM). bf16 input halves the DMA head; loss rel err stays ~1e-6.
Then log-denominator, positives dot, partition-sum matmuls, one scalar out.
"""

import ml_dtypes
import numpy as np

import concourse.bass as bass
import concourse.tile as tile
from concourse import bacc, mybir
from concourse.bass_utils import run_bass_kernel_spmd
from concourse.hw_specs import get_activation_tables
from concourse.masks import make_identity

B = 4096
D = 128
N2 = 2 * B            # 8192 total rows
NCORES = 8
RPC = N2 // NCORES    # 1024 rows per core
MT = RPC // 128       # 8 m-tiles of 128 rows
NCH = N2 // 512       # 16 column chunks of 512
NG = 4                # groups of 4 chunks (2048 cols)
TEMP = 0.5
E2 = float(np.exp(1.0 / TEMP))   # exp(sim_ii / T) with sim_ii == 1

F32 = mybir.dt.float32
F32R = mybir.dt.float32r
BF16 = mybir.dt.bfloat16
AX = mybir.AxisListType
OP = mybir.AluOpType
AF = mybir.ActivationFunctionType

LAST_RESULT = None  # BassKernelResults of the most recent run (for test.py)


def _build_nc():
    nc = bacc.Bacc("TRN2", target_bir_lowering=False)
    xn_d = nc.declare_dram_parameter("xn", [N2, D], BF16, isOutput=False)
    out_d = nc.declare_dram_parameter("out", [1, 1], F32, isOutput=True)

    # Pre-place the one ACT table set that covers both Ln and Exp, so the
    # greedy per-func chooser never inserts mid-kernel table switches.
    table_names = list(get_activation_tables(nc.m.arch).keys())
    combined_id = table_names.index("natural_log_exp_and_others")

    with tile.TileContext(nc) as tc:
        with (
            tc.tile_pool(name="big", bufs=1) as big,
            tc.tile_pool(name="work", bufs=3) as work,
            tc.tile_pool(name="scr", bufs=2) as scr,
            tc.tile_pool(name="ps", bufs=2, space="PSUM") as ps,
        ):
            nc.scalar.add_instruction(mybir.InstLoadActFuncSet(
                name=nc.get_next_instruction_name(), ins=[], outs=[],
                act_func_set_id=combined_id))

            xn_all = big.tile([128, 64, 128], BF16, tag="xn")  # [p, j, d]: row j*128+p
            xhat = big.tile([D, N2], BF16, tag="xhat")         # normalized reps^T
            ns_c = big.tile([128, 64], F32, tag="ns")         # |row|^2 compact
            lnn = big.tile([128, 64], F32, tag="lnn")
            s_c = big.tile([128, 64], F32, tag="s")           # 1/|row| compact
            ones_col = big.tile([128, 1], F32, tag="ones_col")
            rs_all = big.tile([128, MT * NG], F32, tag="rs")  # exp row-sums (m, g)
            ident = big.tile([128, 128], BF16, tag="ident")
            pacc = big.tile([128, 1], F32, tag="pacc")

            def prep_group(g):
                """DMA 4 natural chunks, row norms, 1/norm, scale, transpose
                into xhat columns [2048g, 2048(g+1))."""
                for c in range(4 * g, 4 * g + 4):
                    # alternate the two HWDGE queues (SP / ACT)
                    eng = nc.sync if c % 2 == 0 else nc.scalar
                    eng.dma_start(
                        out=xn_all[:, c * 4:(c + 1) * 4, :],
                        in_=xn_d[c * 512:(c + 1) * 512, :].rearrange(
                            "(t p) d -> p t d", p=128
                        ),
                    )
                    # normsq per 128-row block: fused square + row-sum
                    for j in range(4):
                        jj = c * 4 + j
                        sqs = work.tile([128, 128], F32, tag="sqs")
                        blk = xn_all[:, jj, :]
                        nc.vector.scalar_tensor_tensor(
                            out=sqs, in0=blk, scalar=1.0, in1=blk,
                            op0=OP.mult, op1=OP.mult,
                            accum_out=ns_c[:, jj:jj + 1],
                        )
                # 1/norm = exp(-0.5*ln(normsq)); same ACT table set as exp.
                # group 0 is the latency-critical head: do it per chunk-pair
                # so the chain doesn't wait for all 4 chunk DMAs.
                subs = 2 if g == 0 else 1
                # high priority: these tiny ops must not queue behind the
                # previous quarter's exp stream on ACT (they gate this
                # group's scale->transpose chain and its PSUM slot release)
                with tc.high_priority():
                    for i in range(subs):
                        w = 16 // subs
                        gsl = slice(16 * g + i * w, 16 * g + (i + 1) * w)
                        nc.scalar.activation(
                            out=lnn[:, gsl], in_=ns_c[:, gsl], func=AF.Ln
                        )
                        nc.scalar.activation(
                            out=s_c[:, gsl], in_=lnn[:, gsl], func=AF.Exp,
                            scale=-0.5,
                        )
                # scale rows, PE-transpose into xhat columns (bf16)
                tp = ps.tile([128, 2048], BF16, tag="ps")
                for c in range(4 * g, 4 * g + 4):
                    xsc = work.tile([128, 4, 128], BF16, tag="xsc")
                    nc.vector.tensor_mul(
                        xsc,
                        xn_all[:, c * 4:(c + 1) * 4, :],
                        s_c[:, c * 4:(c + 1) * 4].broadcast_to([128, 4, 128]),
                    )
                    for j in range(4):
                        nc.tensor.transpose(
                            tp[:, (c % 4) * 512 + j * 128:(c % 4) * 512 + (j + 1) * 128],
                            xsc[:, j, :],
                            ident[:],
                        )
                    nc.vector.tensor_copy(
                        xhat[:, c * 512:(c + 1) * 512],
                        tp[:, (c % 4) * 512:(c % 4 + 1) * 512],
                    )
                if g == 2:
                    # positives dot (needs xhat chunks 0,1 and 8,9)
                    prod = scr.tile([128, RPC], F32, tag="scr")
                    nc.vector.scalar_tensor_tensor(
                        out=prod,
                        in0=xhat[:, 0:RPC],
                        scalar=1.0,
                        in1=xhat[:, B:B + RPC],
                        op0=OP.mult,
                        op1=OP.mult,
                        accum_out=pacc,
                    )

            def quarter_half(g, half):
                """4 m-tiles of main work on columns [2048g, 2048(g+1))."""
                for m in range(4 * half, 4 * half + 4):
                    pst = ps.tile([128, 2048], F32, tag="ps")
                    lhsT = xhat[:, m * 128:(m + 1) * 128]
                    for s4 in range(4):
                        col = g * 2048 + s4 * 512
                        nc.tensor.matmul(
                            pst[:, s4 * 512:(s4 + 1) * 512],
                            lhsT=lhsT,
                            rhs=xhat[:, col:col + 512],
                            start=True,
                            stop=True,
                        )
                    sc = scr.tile([128, 2048], BF16, tag="scr")
                    nc.scalar.activation(
                        out=sc,
                        in_=pst,
                        func=AF.Exp,
                        scale=1.0 / TEMP,
                        accum_out=rs_all[:, m * NG + g:m * NG + g + 1],
                    )

            nc.vector.memset(ones_col, 1.0)
            make_identity(nc, ident[:])

            # interleave: group g+1 prep emitted mid-quarter-g so its DMAs,
            # DVE work and PSUM slot use hide under the ACT exp stream
            prep_group(0)
            quarter_half(0, 0)
            prep_group(1)
            quarter_half(0, 1)
            quarter_half(1, 0)
            prep_group(2)
            quarter_half(1, 1)
            quarter_half(2, 0)
            prep_group(3)
            quarter_half(2, 1)
            quarter_half(3, 0)
            quarter_half(3, 1)

            # ---- finals ----
            rowsum = big.tile([128, MT], F32, tag="rowsum")
            nc.vector.tensor_reduce(
                out=rowsum,
                in_=rs_all[:].rearrange("p (m g) -> p m g", g=NG),
                axis=AX.X,
                op=OP.add,
            )
            den = big.tile([128, MT], F32, tag="den")
            nc.vector.tensor_scalar_add(out=den, in0=rowsum, scalar1=-E2)
            logden = big.tile([128, MT], F32, tag="logden")
            nc.scalar.activation(out=logden, in_=den, func=AF.Ln)
            ldps = ps.tile([1, MT], F32, tag="ps")
            nc.tensor.matmul(ldps, lhsT=ones_col, rhs=logden, start=True, stop=True)
            pps = ps.tile([1, 1], F32, tag="ps")
            nc.tensor.matmul(pps, lhsT=ones_col, rhs=pacc, start=True, stop=True)

            l1 = big.tile([1, 1], F32, tag="l1")
            nc.vector.tensor_reduce(out=l1, in_=ldps, axis=AX.X, op=OP.add)
            t2 = big.tile([1, 1], F32, tag="t2")
            nc.vector.tensor_scalar_mul(out=t2, in0=pps, scalar1=-2.0)
            res = big.tile([1, 1], F32, tag="res")
            nc.vector.tensor_add(res, l1, t2)
            nc.vector.tensor_scalar_mul(out=res, in0=res, scalar1=1.0 / N2)
            nc.sync.dma_start(out=out_d[:, :], in_=res)

    nc.compile()
    return nc


_NC = None


def kernel(proj_1: np.ndarray, proj_2: np.ndarray) -> np.ndarray:
    global _NC, LAST_RESULT
    import os

    reps = np.concatenate(
        [np.asarray(proj_1, np.float32), np.asarray(proj_2, np.float32)], axis=0
    )
    assert reps.shape == (N2, D)

    in_maps = [
        {"xn": np.ascontiguousarray(np.roll(reps, -c * RPC, axis=0)).astype(ml_dtypes.bfloat16)}
        for c in range(NCORES)
    ]

    if _NC is None:
        _NC = _build_nc()

    trace = bool(os.environ.get("CONTRASTIVE_TRACE"))
    result = run_bass_kernel_spmd(
        _NC, in_maps, core_ids=list(range(NCORES)), trace=trace
    )
    LAST_RESULT = result
    total = sum(float(r["out"][0, 0]) for r in result.results)
    return np.float32(total)
